# revision 1
# baseline (speedup 1.0000x reference)
"""AUAvULoss kernel for Trainium2, data-parallel over N across 8 NeuronCores.

Two SPMD launches:
  K1: streams probs/y/weights (12 MB/core) in 1MB chunks over two HWDGE rings
      (sync+scalar), computes per-sample entropy (unc, stored as bf16 hi+lo
      pair), quadrant weight arrays A/P/B/Q (bf16), partial CE/focal sums
      (fused scalar_tensor_tensor / activation accumulators) and per-core
      min/max of unc.  Work is balanced across DVE / Pool / ACT.
  host: all-reduce min/max -> 21 thresholds.
  K2: transpose-DMAs the per-sample arrays into sample-on-partition layout
      [128, 1024], generates LE masks (unc <= th_t, 21 per sample) with
      broadcast-AP compares on DVE, then contracts on the PE:
      per 128-sample chunk  lhsT=W4[128,4] (strided AP), rhs=LE[128,21],
      accumulated into PSUM with 4-way tile_position column packing.
  host: prefix algebra -> n_ac/n_au/n_ic/n_iu -> AvU AUC -> losses.
"""

import sys
from contextlib import ExitStack

import numpy as np

for _p in ("/opt/trn_rl_repo",):
    if _p not in sys.path:
        sys.path.insert(0, _p)

import concourse.bacc as bacc
import concourse.bass as bass
import concourse.mybir as mybir
import concourse.tile as tile
from concourse.bass_utils import run_bass_kernel_spmd

f32 = mybir.dt.float32
bf16 = mybir.dt.bfloat16
AF = mybir.ActivationFunctionType
OP = mybir.AluOpType
AX = mybir.AxisListType

NCORES = 8
N, C = 1_000_000, 8
R = N // NCORES          # 125_000 rows per core
P = 125                  # SBUF partitions used in K1
W = R // P               # 1000 rows per partition
NCH = 4                  # K1 chunks (1 MB per tensor per chunk)
CW = W // NCH            # 250 rows/partition per chunk
CE_W = CW * C            # 2000 elements/partition per chunk
NTH = 21
PADN = 131072            # 1024*128, K2 transposed layout size
NCHK = PADN // 128       # 1024 sample chunks in K2
GB = 64                  # LE-gen batch: chunks per DVE op
EPS = 1e-10
BETA = 1.0


def build_k1(label_col):
    """label_col: class index in [0,8) that predictions are compared to,
    or None (labels scalar out of range -> nothing is 'correct')."""
    nc = bacc.Bacc("TRN2", target_bir_lowering=False, debug=False,
                   enable_asserts=False, num_devices=NCORES)
    pr_d = nc.dram_tensor("probs", [R, C], f32, kind="ExternalInput").ap()
    y_d = nc.dram_tensor("y", [R, C], f32, kind="ExternalInput").ap()
    w_d = nc.dram_tensor("w", [R, C], f32, kind="ExternalInput").ap()

    outs = {}
    for nm in ("uh", "ul", "Aw", "Pw", "Bw", "Qw"):
        outs[nm] = nc.dram_tensor(nm, [PADN], bf16, kind="ExternalOutput").ap()
    ce_d = nc.dram_tensor("ce", [P, NCH], f32, kind="ExternalOutput").ap()
    fo_d = nc.dram_tensor("fo", [P, NCH], f32, kind="ExternalOutput").ap()
    ex_d = nc.dram_tensor("ex", [P, 2], f32, kind="ExternalOutput").ap()

    pr_r = pr_d.rearrange("(p w) c -> p (w c)", p=P)   # [125, 8000]
    y_r = y_d.rearrange("(p w) c -> p (w c)", p=P)
    w_r = w_d.rearrange("(p w) c -> p (w c)", p=P)

    with tile.TileContext(nc) as tc, ExitStack() as ctx:
        io = ctx.enter_context(tc.tile_pool(name="io", bufs=2))
        sc = ctx.enter_context(tc.tile_pool(name="sc", bufs=2))
        ps = ctx.enter_context(tc.tile_pool(name="ps", bufs=1))

        unc_t = ps.tile([P, W], f32, tag="unc")
        conf_t = ps.tile([P, W], f32, tag="conf")
        corr_t = ps.tile([P, W], f32, tag="corr")
        tanh_t = ps.tile([P, W], f32, tag="tanh")
        ce_acc = ps.tile([P, NCH], f32, tag="ceacc")
        fo_acc = ps.tile([P, NCH], f32, tag="foacc")
        ex_t = ps.tile([P, 2], f32, tag="ex")
        ones_t = ps.tile([P, 1], f32, tag="ones")
        nc.vector.memset(ones_t[:], 1.0)

        for k in range(NCH):
            sl = bass.ts(k, CE_W)
            pr = io.tile([P, CE_W], f32, tag="pr")
            nc.sync.dma_start(pr[:], pr_r[:, sl])
            ww = io.tile([P, CE_W], f32, tag="ww")
            nc.sync.dma_start(ww[:], w_r[:, sl])
            yy = io.tile([P, CE_W], f32, tag="yy")
            nc.scalar.dma_start(yy[:], y_r[:, sl])

            lg = sc.tile([P, CE_W], f32, tag="lg")
            nc.scalar.activation(lg[:], pr[:], AF.Ln)

            pl = sc.tile([P, CE_W], f32, tag="pl")
            nc.gpsimd.tensor_tensor(pl[:], pr[:], lg[:], op=OP.mult)

            pr3 = pr[:].rearrange("p (a c) -> p a c", c=C)
            pl3 = pl[:].rearrange("p (a c) -> p a c", c=C)
            ksl = bass.ts(k, CW)
            nc.vector.tensor_reduce(unc_t[:, ksl], pl3, axis=AX.X,
                                    op=OP.add, negate=True)
            nc.vector.tensor_reduce(conf_t[:, ksl], pr3, axis=AX.X, op=OP.max)

            t1 = sc.tile([P, CE_W], f32, tag="t1")
            if k % 2 == 0:
                nc.gpsimd.tensor_tensor(t1[:], yy[:], lg[:], op=OP.mult)
            else:
                nc.vector.tensor_tensor(t1[:], yy[:], lg[:], op=OP.mult)
            junka = sc.tile([P, CE_W], f32, tag="junka")
            nc.scalar.activation(junka[:], t1[:], AF.Copy,
                                 accum_out=ce_acc[:, k:k + 1])
            junkb = sc.tile([P, CE_W], f32, tag="junkb")
            nc.vector.scalar_tensor_tensor(
                out=junkb[:], in0=t1[:], scalar=ones_t[:, 0:1], in1=ww[:],
                op0=OP.mult, op1=OP.mult, accum_out=fo_acc[:, k:k + 1])

            if label_col is not None:
                prL = pr3[:, :, label_col:label_col + 1]
                prL = prL.rearrange("p a c -> p (a c)")
                nc.vector.tensor_tensor(corr_t[:, ksl], prL,
                                        conf_t[:, ksl], op=OP.is_ge)
            else:
                nc.vector.memset(corr_t[:, ksl], 0.0)

        nc.scalar.activation(tanh_t[:], unc_t[:], AF.Tanh)

        bfo = {nm: ps.tile([P, W], bf16, tag="o" + nm, name="o" + nm)
               for nm in ("uh", "ul", "Aw", "Pw", "Bw", "Qw")}
        uh32 = sc.tile([P, W], f32, tag="uh32")
        nc.vector.tensor_copy(bfo["uh"][:], unc_t[:])
        nc.vector.tensor_copy(uh32[:], bfo["uh"][:])
        nc.vector.tensor_tensor(bfo["ul"][:], unc_t[:], uh32[:],
                                op=OP.subtract)
        nc.vector.tensor_tensor(bfo["Aw"][:], conf_t[:], corr_t[:],
                                op=OP.mult)
        pt = sc.tile([P, W], f32, tag="pt")
        nc.vector.tensor_tensor(pt[:], conf_t[:], tanh_t[:], op=OP.mult)
        nc.vector.tensor_tensor(bfo["Pw"][:], pt[:], corr_t[:], op=OP.mult)
        s1 = sc.tile([P, W], f32, tag="s1")
        nc.vector.tensor_scalar_add(s1[:], conf_t[:], -1.0)
        s2 = sc.tile([P, W], f32, tag="s2")
        nc.vector.tensor_scalar_add(s2[:], corr_t[:], -1.0)
        nc.vector.tensor_tensor(bfo["Bw"][:], s1[:], s2[:], op=OP.mult)
        qt = sc.tile([P, W], f32, tag="qt")
        nc.vector.tensor_tensor(qt[:], s1[:], tanh_t[:], op=OP.mult)
        nc.vector.tensor_tensor(bfo["Qw"][:], qt[:], s2[:], op=OP.mult)

        nc.vector.tensor_reduce(ex_t[:, 0:1], unc_t[:], axis=AX.X, op=OP.min)
        nc.vector.tensor_reduce(ex_t[:, 1:2], unc_t[:], axis=AX.X, op=OP.max)

        zt = ps.tile([1, PADN - R], bf16, tag="zt")
        nc.gpsimd.memset(zt[:], 0.0)
        for j, nm in enumerate(("uh", "ul", "Aw", "Pw", "Bw", "Qw")):
            eng = nc.sync if j % 2 == 0 else nc.scalar
            main = outs[nm][0:R].rearrange("(p w) -> p w", p=P)
            eng.dma_start(main, bfo[nm][:])
            tail = outs[nm][R:PADN].rearrange("(a b) -> a b", a=1)
            eng.dma_start(tail, zt[:])
        nc.sync.dma_start(ce_d[:, :], ce_acc[:])
        nc.scalar.dma_start(fo_d[:, :], fo_acc[:])
        nc.sync.dma_start(ex_d[:, :], ex_t[:])

    nc.compile()
    return nc


def build_k2():
    nc = bacc.Bacc("TRN2", target_bir_lowering=False, debug=False,
                   enable_asserts=False, num_devices=NCORES)
    ins = {}
    for nm in ("uh", "ul", "Aw", "Pw", "Bw", "Qw"):
        ins[nm] = nc.dram_tensor(nm, [PADN], bf16, kind="ExternalInput").ap()
    th_d = nc.dram_tensor("th", [128, NTH], f32, kind="ExternalInput").ap()
    acc_d = nc.dram_tensor("acc", [128, NTH], f32, kind="ExternalOutput").ap()

    with tile.TileContext(nc) as tc, ExitStack() as ctx:
        pp = ctx.enter_context(tc.tile_pool(name="pp", bufs=1))
        lp = ctx.enter_context(tc.tile_pool(name="lp", bufs=3))
        psp = ctx.enter_context(tc.tile_pool(name="psp", bufs=1, space="PSUM"))

        uh_t = pp.tile([128, NCHK], bf16, tag="uh")
        nc.sync.dma_start_transpose(
            uh_t[:], ins["uh"].rearrange("(a b) -> a b", b=128))
        ul_t = pp.tile([128, NCHK], bf16, tag="ul")
        nc.scalar.dma_start_transpose(
            ul_t[:], ins["ul"].rearrange("(a b) -> a b", b=128))
        w4_t = pp.tile([128, 4 * NCHK], bf16, tag="w4")
        for q, nm in enumerate(("Aw", "Pw", "Bw", "Qw")):
            eng = nc.sync if q % 2 == 0 else nc.scalar
            eng.dma_start_transpose(
                w4_t[:, q * NCHK:(q + 1) * NCHK],
                ins[nm].rearrange("(a b) -> a b", b=128))
        th_t = pp.tile([128, NTH], f32, tag="th")
        nc.sync.dma_start(th_t[:], th_d[:, :])

        uncT = pp.tile([128, NCHK], f32, tag="uncT")
        nc.vector.tensor_tensor(uncT[:], uh_t[:], ul_t[:], op=OP.add)

        psum = psp.tile([128, NTH], f32, tag="acc")
        nc.vector.memset(psum[:], 0.0)

        w4v = w4_t[:].rearrange("p (q c) -> p c q", c=NCHK)
        thb = th_t[:].unsqueeze(1).broadcast_to((128, GB, NTH))
        for g in range(NCHK // GB):
            le = lp.tile([128, GB * NTH], bf16, tag="le")
            ub = uncT[:, g * GB:(g + 1) * GB].unsqueeze(2)
            ub = ub.broadcast_to((128, GB, NTH))
            nc.vector.tensor_tensor(
                le[:].rearrange("p (g t) -> p g t", t=NTH),
                thb, ub, op=OP.is_ge)
            for j in range(GB):
                c = g * GB + j
                i = c % 4
                lhsT = w4v[:, c:c + 1, :].rearrange("p c q -> p (c q)")
                rhs = le[:, j * NTH:(j + 1) * NTH]
                nc.tensor.matmul(psum[32 * i:32 * i + 4, :], lhsT, rhs,
                                 start=False, stop=(c >= NCHK - 4),
                                 skip_group_check=True,
                                 tile_position=(0, 32 * i))

        sb = pp.tile([128, NTH], f32, tag="sb")
        nc.vector.tensor_copy(sb[:], psum[:])
        nc.sync.dma_start(acc_d[:, :], sb[:])

    nc.compile()
    return nc


_cache = {}


def _get_k1(label_col):
    key = ("k1", label_col)
    if key not in _cache:
        _cache[key] = build_k1(label_col)
    return _cache[key]


def _get_k2():
    if "k2" not in _cache:
        _cache["k2"] = build_k2()
    return _cache["k2"]


def kernel(probs, y, weights, _results=None, _trace=False):
    probs = np.ascontiguousarray(probs, dtype=np.float32)
    y = np.ascontiguousarray(y, dtype=np.float32)
    weights = np.ascontiguousarray(weights, dtype=np.float32)

    flat_label = int(np.argmax(y))
    label_col = flat_label if flat_label < C else None

    nc1 = _get_k1(label_col)
    in1 = [{"probs": probs[i * R:(i + 1) * R],
            "y": y[i * R:(i + 1) * R],
            "w": weights[i * R:(i + 1) * R]} for i in range(NCORES)]
    tr1 = {"trace": True, "tmpdir": "/tmp/trace_k1"} if _trace else {}
    if _trace:
        import os as _os
        import shutil as _sh
        for d in ("/tmp/trace_k1", "/tmp/trace_k2"):
            _sh.rmtree(d, ignore_errors=True)
            _os.makedirs(d, exist_ok=True)
    r1 = run_bass_kernel_spmd(nc1, in1, core_ids=list(range(NCORES)), **tr1)
    outs1 = r1.results

    ce_sum = sum(float(o["ce"].sum(dtype=np.float64)) for o in outs1)
    fo_sum = sum(float(o["fo"].sum(dtype=np.float64)) for o in outs1)
    CE_loss = -ce_sum / N
    focal_loss = -fo_sum / N
    umin = min(float(o["ex"][:, 0].min()) for o in outs1)
    umax = max(float(o["ex"][:, 1].max()) for o in outs1)
    SP = sum(float(o["Pw"].astype(np.float64).sum()) for o in outs1)
    SQ = sum(float(o["Qw"].astype(np.float64).sum()) for o in outs1)

    th01 = np.linspace(0.0, 1.0, NTH).astype(np.float32)
    unc_th = (np.float32(umin) + th01 *
              (np.float32(umax) - np.float32(umin))).astype(np.float32)
    th_b = np.broadcast_to(unc_th, (128, NTH)).copy()

    nc2 = _get_k2()
    in2 = [{"uh": o["uh"], "ul": o["ul"], "Aw": o["Aw"], "Pw": o["Pw"],
            "Bw": o["Bw"], "Qw": o["Qw"], "th": th_b} for o in outs1]
    tr2 = {"trace": True, "tmpdir": "/tmp/trace_k2"} if _trace else {}
    r2 = run_bass_kernel_spmd(nc2, in2, core_ids=list(range(NCORES)), **tr2)
    outs2 = r2.results

    S_le = np.zeros((4, NTH), dtype=np.float64)  # A, P, B, Q
    for o in outs2:
        a = o["acc"].astype(np.float64)
        for i in range(4):
            S_le += a[32 * i:32 * i + 4, :]

    n_ac = S_le[0] - S_le[1]
    n_au = SP - S_le[1]
    n_ic = S_le[2] - S_le[3]
    n_iu = SQ - S_le[3]
    avu = (n_ac + n_iu) / (n_ac + n_au + n_ic + n_iu + EPS)
    dx = np.diff(th01.astype(np.float64))
    auc_avu = float(np.sum(0.5 * (avu[1:] + avu[:-1]) * dx))
    avu_loss = -BETA * np.log(auc_avu + EPS) + focal_loss

    if _results is not None:
        _results.update(r1=r1, r2=r2, umin=umin, umax=umax, n=np.stack(
            [n_ac, n_au, n_ic, n_iu]), avu=avu, auc=auc_avu)
    return (np.float32(avu_loss), np.float32(CE_loss))

